# revision 39
# baseline (speedup 1.0000x reference)
"""Causal self-attention (b=2, n=2048, d=1024, 16 heads) on 8 NeuronCores.

Sharding: core c handles batch b = c // 4 and head group g4 = c % 4
(heads 4*g4..4*g4+3).  qkv weights column-sharded, proj weights row-sharded
(Megatron); each core emits a partial [2048, 1024] proj output and the
host sums the 4 partials per batch (b_proj added host-side).

Engine plan (per core, cost-model driven):
  q/k/v projection : fp8e4m3 DoubleRow with residual compensation:
                     x -> f8(4x) + f8(residual), W -> f8(256W) +
                     f8(residual); acc = x8@W8 + xr8@W8 + x8@Wr8 in one
                     PSUM group (all terms share the 1024x scale), giving
                     ~0.2% error at 1.5x the fp8 matmul cost.
  S = K^T Q        : bf16 (q/k fp8 would put ~1.5%/quantization of
                     correlated tilt into softmax - too close to the 2e-2
                     gate), heads at partition bases 0/64, exact causal
                     triangle at 128-col granularity; exp folds the 1/8
                     softmax scale and the 2^-20 projection scale.
  exp              : ACT, PSUM f32 -> SBUF bf16.
  causal mask      : gpsimd affine_select on the diagonal et blocks.
  A @ V            : flipped: et [128k,128q] stationary (bf16), [V|1]
                     [128k,65] moving -> token-major o [128q,65] with the
                     softmax denominator in column 64.  Four 65-col
                     accumulation regions share one PSUM bank: only the
                     first matmul into the bank uses start=True (bank-wide
                     pending-zero), later regions' first matmul overwrites
                     via the pending-zero bits.
  normalize        : DVE reciprocal of col 64 + per-(token,head) multiply,
                     output bf16 token-major.
  transpose        : PE is_transpose (bf16, identity rhs) back to
                     feature-major onT for the output projection.
  projection       : bf16 onT stationary x bf16 Wp moving; f32 y out.

Scheduling: PE work (~92us) exceeds ACT exp (~77us), so QK->exp supply is
first-class; V/qkproj(j+1)/norm units drain at a fixed slot rate while
deferrable proj units pop only when the quarter's emitted exp time exceeds
its emitted PE time (leftovers carry to the next quarter / tail).
"""
import sys

sys.path.insert(0, "/opt/trn_rl_repo")

import numpy as np

import concourse.bass as bass  # noqa: F401
import concourse.mybir as mybir
import concourse.tile as tile
from concourse import bacc
from concourse.bass_utils import run_bass_kernel_spmd

F32 = mybir.dt.float32
F32R = mybir.dt.float32r
BF16 = mybir.dt.bfloat16
FP8 = mybir.dt.float8e4
Exp = mybir.ActivationFunctionType.Exp
DR = mybir.MatmulPerfMode.DoubleRow

B = 2
N = 2048
D = 1024
NH = 16
HD = 64
NCORES = 8
GROUPS = 4                # head groups (cores per batch)
HPC = NH // GROUPS        # heads per core = 4
QS = 512                  # q_super width
NQS = N // QS             # 4
NB = N // 128             # 16 token blocks
CCH = D // 128            # 8 contraction chunks
KP = CCH // 2             # 4 DoubleRow contraction pairs
XS = 4.0                  # fp8 scale on x
WS = 256.0                # fp8 scale on W_qkv
EXP_SCALE = 0.125 / (XS * WS) ** 2     # q,k both carry the 1024x scale
VSCALE = 1.0 / (XS * WS)               # V drain rescale

_CACHE = {}


def _build():
    nc = bacc.Bacc("TRN2", target_bir_lowering=False, debug=False,
                   num_devices=NCORES)
    x8d = nc.dram_tensor("x8", [D, N], FP8, kind="ExternalInput").ap()
    xr8d = nc.dram_tensor("xr8", [D, N], FP8, kind="ExternalInput").ap()
    W8d = nc.dram_tensor("W8", [128, KP * 2 * 4 * 128], FP8,
                         kind="ExternalInput").ap()
    Wr8d = nc.dram_tensor("Wr8", [128, KP * 2 * 4 * 128], FP8,
                          kind="ExternalInput").ap()
    Wv8d = nc.dram_tensor("Wv8", [128, KP * 2 * 256], FP8,
                          kind="ExternalInput").ap()
    Wvr8d = nc.dram_tensor("Wvr8", [128, KP * 2 * 256], FP8,
                           kind="ExternalInput").ap()
    Wp = nc.dram_tensor("Wp", [128, 2 * D], BF16, kind="ExternalInput").ap()
    bqk = nc.dram_tensor("bqk", [128, 4], F32, kind="ExternalInput").ap()
    vbias = nc.dram_tensor("vbias", [128, 256], F32, kind="ExternalInput").ap()
    identD = nc.dram_tensor("identD", [128, 128], BF16, kind="ExternalInput").ap()
    y = nc.dram_tensor("y", [N, D], BF16, kind="ExternalOutput").ap()

    with tile.TileContext(nc) as tc:
        with (
            tc.tile_pool(name="persist", bufs=1) as pp,
            tc.tile_pool(name="x8_pool", bufs=2) as x8_pool,
            tc.tile_pool(name="xr8_pool", bufs=2) as xr8_pool,
            tc.tile_pool(name="et_pool", bufs=8) as et_pool,
            tc.tile_pool(name="onorm_pool", bufs=2) as onorm_pool,
            tc.tile_pool(name="work", bufs=4) as work,
            tc.tile_pool(name="ysb_pool", bufs=6) as ysb_pool,
            tc.tile_pool(name="mm", bufs=2, space="PSUM") as mm,
            tc.tile_pool(name="spool", bufs=2, space="PSUM") as spool,
            tc.tile_pool(name="opool", bufs=2, space="PSUM") as opool,
        ):
            # ---- persistent tiles ----
            W8_sb = pp.tile([128, KP, 2, 4, 128], FP8)    # (kp, two, tile, m)
            Wr8_sb = pp.tile([128, KP, 2, 4, 128], FP8)
            Wv8_sb = pp.tile([128, KP, 2, 256], FP8)
            Wvr8_sb = pp.tile([128, KP, 2, 256], FP8)
            Wp_sb = pp.tile([128, 2, D], BF16)
            bqk_sb = pp.tile([128, 4], F32)               # per (qk,g) tile
            vb_sb = pp.tile([128, 256], F32)
            ident = pp.tile([128, 128], BF16)
            # q/k bf16 (1024x scaled), heads paired at bases 0/64
            qkT = {g: pp.tile([128, 2, N], BF16, name=f"qkT_{g}")
                   for g in range(2)}
            vaug = pp.tile([128, NB, HPC, 65], BF16)      # [V | 1] token-major
            onT = pp.tile([128, 2, N], BF16)              # feature-major o

            W8_r = W8d.rearrange("p (kp two t m) -> p kp two t m",
                                 kp=KP, two=2, t=4)
            Wr8_r = Wr8d.rearrange("p (kp two t m) -> p kp two t m",
                                   kp=KP, two=2, t=4)
            Wv8_r = Wv8d.rearrange("p (kp two f) -> p kp two f", kp=KP, two=2)
            Wvr8_r = Wvr8d.rearrange("p (kp two f) -> p kp two f",
                                     kp=KP, two=2)
            x8_r = x8d.rearrange("(kp two p) n -> p kp two n", p=128, two=2)
            xr8_r = xr8d.rearrange("(kp two p) n -> p kp two n", p=128, two=2)
            Wp_r = Wp.rearrange("p (c f) -> p c f", c=2)
            y_r = y.rearrange("(t p) f -> t p f", p=128)

            def fetch_x(q):
                t0, t1 = QS * q, QS * (q + 1)
                x8q = x8_pool.tile([128, KP, 2, QS], FP8, tag="x8",
                                   name=f"x8_{q}")
                nc.sync.dma_start(x8q[:], x8_r[:, :, :, t0:t1])
                xr8q = xr8_pool.tile([128, KP, 2, QS], FP8, tag="xr8",
                                     name=f"xr8_{q}")
                nc.sync.dma_start(xr8q[:], xr8_r[:, :, :, t0:t1])
                return x8q, xr8q

            # ---------- per-quarter state ----------
            onorm_sh = {}      # (j, g) -> onorm tile (shared across halves)
            pending = []       # from quarter j-1: [norm_g1, transp_g1] + proj
            carry = []         # deferrable units carried across quarters
            next_x = None
            # pacing state (reset per quarter): ns of exp emitted vs ns of
            # PE emitted
            bal = {"act": 0.0, "pe": 0.0}

            def pe_note(ns):
                bal["pe"] += ns

            for j in range(NQS):
                ts, te = QS * j, QS * (j + 1)
                n_i = 4 * j + 4
                bal["act"] = 0.0
                bal["pe"] = 0.0

                # ---- input DMAs, ordered by first consumption ----
                if j == 0:
                    # quarter-0 critical path: W8/Wr8 first, then per-kp
                    # x8/xr8 chunks so qkproj tracks chunk arrivals
                    nc.sync.dma_start(W8_sb[:], W8_r)
                    nc.sync.dma_start(Wr8_sb[:], Wr8_r)
                    nc.sync.dma_start(bqk_sb[:], bqk)
                    x8q = x8_pool.tile([128, KP, 2, QS], FP8, tag="x8",
                                       name="x8_0")
                    xr8q = xr8_pool.tile([128, KP, 2, QS], FP8, tag="xr8",
                                         name="xr8_0")
                    for kp in range(KP):
                        nc.sync.dma_start(x8q[:, kp, :, :],
                                          x8_r[:, kp, :, 0:QS])
                        nc.sync.dma_start(xr8q[:, kp, :, :],
                                          xr8_r[:, kp, :, 0:QS])
                    nc.sync.dma_start(Wv8_sb[:], Wv8_r)
                    nc.sync.dma_start(Wvr8_sb[:], Wvr8_r)
                    nc.sync.dma_start(vb_sb[:], vbias)
                    nc.sync.dma_start(ident[:], identD)
                    nc.sync.dma_start(Wp_sb[:], Wp_r)
                    # ones columns of [V|1] via DVE (0*x + 1)
                    nc.vector.tensor_scalar(
                        out=vaug[:, :, :, 64],
                        in0=ident[:, 0:64].rearrange("p (a b) -> p a b", b=HPC),
                        scalar1=0.0,
                        scalar2=1.0,
                        op0=mybir.AluOpType.mult,
                        op1=mybir.AluOpType.add,
                    )
                    next_x = fetch_x(1)
                else:
                    x8q, xr8q = next_x
                    if j + 1 < NQS:
                        next_x = fetch_x(j + 1)

                # ---- q/k projection: tile t = 2*qk + g = 128 features of
                # heads {2g, 2g+1}; three DoubleRow chains share one PSUM
                # accumulation (all at the 1024x scale).
                def make_qkproj_units(jq, xs_, xrs_):
                    tsq, teq = QS * jq, QS * (jq + 1)
                    pss = {}
                    chains = [(W8_sb, xs_), (W8_sb, xrs_), (Wr8_sb, xs_)]

                    def unit(t, ch):
                        def emit():
                            if ch == 0:
                                pss[t] = mm.tile([128, QS], F32, tag="mm",
                                                 name=f"qk{jq}{t}")
                            ps = pss[t]
                            Wt, xt = chains[ch]
                            for kp in range(KP):
                                nc.tensor.matmul(
                                    ps[:],
                                    Wt[:, kp, :, t, :],
                                    xt[:, kp, :, :],
                                    start=(ch == 0 and kp == 0),
                                    stop=(ch == 2 and kp == KP - 1),
                                    perf_mode=DR,
                                )
                            pe_note(4 * 107)
                            if ch == 2:
                                nc.vector.tensor_scalar_add(
                                    qkT[t % 2][:, t // 2, tsq:teq],
                                    ps[:],
                                    bqk_sb[:, t : t + 1],
                                )
                        return emit
                    return [unit(t, ch) for t in (0, 2, 1, 3)
                            for ch in range(3)]

                if j == 0:
                    # inline: g0's tiles first, kp-outer so matmuls track
                    # the x8/xr8 chunk DMAs
                    u0 = make_qkproj_units(0, x8q, xr8q)
                    for u_ in u0[:6]:
                        u_()
                    extra_qk0 = u0[6:]
                else:
                    extra_qk0 = []

                # ---- previous quarter's g1 norm chain ----
                for u_ in pending[:2]:
                    u_()
                nextq = (make_qkproj_units(j + 1, *next_x)
                         if j + 1 < NQS else [])
                # must-run-this-quarter fillers (slot-rate paced)
                queue = extra_qk0 + nextq
                # deferrable fillers (budget paced): carried + prev proj
                defq = carry + pending[2:]
                pending = []
                carry = []

                # ---- V units: fp8 DR compensated, token-major out ----
                def make_v_units(q=j, xs_=x8q, xrs_=xr8q):
                    vch = [(xs_, Wv8_sb), (xrs_, Wv8_sb), (xs_, Wvr8_sb)]

                    def unit(blk):
                        def emit():
                            tb = 4 * q + blk
                            vps = mm.tile([128, 256], F32, tag="mm",
                                          name=f"v{q}{blk}")
                            for ch in range(3):
                                xt, Wt = vch[ch]
                                for kp in range(KP):
                                    nc.tensor.matmul(
                                        vps[:],
                                        xt[:, kp, :,
                                           128 * blk : 128 * (blk + 1)],
                                        Wt[:, kp, :, :],
                                        start=(ch == 0 and kp == 0),
                                        stop=(ch == 2 and kp == KP - 1),
                                        perf_mode=DR,
                                    )
                            pe_note(12 * 53)
                            # vaug = vps * 2^-10 + vbias, bf16
                            nc.vector.scalar_tensor_tensor(
                                out=vaug[:, tb, :, 0:64],
                                in0=vps.rearrange("p (h c) -> p h c", c=64),
                                scalar=VSCALE,
                                in1=vb_sb.rearrange("p (h c) -> p h c", c=64),
                                op0=mybir.AluOpType.mult,
                                op1=mybir.AluOpType.add,
                            )
                        return emit
                    return [unit(blk) for blk in range(4)]

                # ---- output projection units for quarter j (deferrable,
                # one matmul per unit) ----
                def make_proj(jj=j):
                    tail = jj == NQS - 1
                    ysbs = {}
                    ypss = {}

                    def unit(blk, nh, c):
                        def emit():
                            tb = 4 * jj + blk
                            if c == 0:
                                ypss[(tb, nh)] = mm.tile(
                                    [128, QS], F32, tag="mm",
                                    name=f"y{tb}{nh}")
                            yps = ypss[(tb, nh)]
                            nc.tensor.matmul(
                                yps[:],
                                onT[:, c, 128 * tb : 128 * (tb + 1)],
                                Wp_sb[:, c, QS * nh : QS * (nh + 1)],
                                start=(c == 0),
                                stop=(c == 1),
                            )
                            if c == 0:
                                return 213
                            if nh == 0:
                                ysbs[tb] = ysb_pool.tile(
                                    [128, 2, QS], BF16, tag="ysb", bufs=6,
                                    name=f"ysb{tb}")
                            ysb = ysbs[tb]
                            if tail and (blk + nh) % 2 == 1:
                                nc.scalar.copy(ysb[:, nh, :], yps[:])
                            else:
                                nc.vector.tensor_copy(ysb[:, nh, :], yps[:])
                            if nh == 1:
                                nc.sync.dma_start(
                                    y_r[tb], ysb.rearrange("p a b -> p (a b)"))
                            return 213
                        return emit
                    return unit

                pu = make_proj()

                v_units = make_v_units()

                # ---- attention for q_super j, per head pair g ----
                for g in range(2):
                    o_ps = {
                        half: opool.tile([128, 2, 2, 65], F32, tag="o",
                                         name=f"o{j}{g}{half}")
                        for half in range(2)
                    }
                    first_touch = {half: True for half in range(2)}
                    ets = {}

                    def emit_qk(i, g=g, j=j, ets=ets):
                        t = i - 4 * j
                        qs0 = 128 * t if t >= 0 else 0
                        sps = spool.tile([128, 2, QS], F32, tag="s",
                                         name=f"s{j}{g}{i}")
                        for l in range(2):
                            nc.tensor.matmul(
                                sps[:, l, qs0:],
                                qkT[g][64 * l : 64 * (l + 1), 1,
                                       128 * i : 128 * (i + 1)],
                                qkT[g][64 * l : 64 * (l + 1), 0,
                                       QS * j + qs0 : QS * (j + 1)],
                                start=True,
                                stop=True,
                            )
                        et = et_pool.tile([128, 2, QS], BF16, tag="et",
                                          name=f"et{j}{g}{i}")
                        nc.scalar.activation(
                            et[:, :, qs0:], sps[:, :, qs0:], Exp,
                            scale=EXP_SCALE,
                        )
                        cols = QS - qs0
                        bal["act"] += 2 * cols * 0.8333 + 217
                        bal["pe"] += 2 * cols * 0.4167
                        if t >= 0:
                            # mask the diagonal 128-block on gpsimd
                            nc.gpsimd.affine_select(
                                out=et[:, :, qs0 : qs0 + 128],
                                in_=et[:, :, qs0 : qs0 + 128],
                                compare_op=mybir.AluOpType.is_ge,
                                fill=0.0,
                                base=0,
                                pattern=[[0, 2], [1, 128]],
                                channel_multiplier=-1,
                            )
                        ets[i] = et

                    def emit_av(i, g=g, j=j, ets=ets, o_ps=o_ps,
                                first_touch=first_touch):
                        t = i - 4 * j
                        if g == 0 and t >= 0:
                            # diagonal AV(i) consumes vaug[4j+t]: flush V
                            while len(v_units) > 3 - t:
                                v_units.pop(0)()
                        et = ets.pop(i)
                        for u in range(max(0, t), 4):
                            half, u2 = divmod(u, 2)
                            for l in range(2):
                                st = first_touch[half]
                                first_touch[half] = False
                                nc.tensor.matmul(
                                    o_ps[half][:, l, u2, :],
                                    et[:, l, 128 * u : 128 * (u + 1)],
                                    vaug[:, i, 2 * g + l, :],
                                    start=st,
                                    stop=(i == 4 * j + u),
                                    skip_group_check=True,
                                )
                        bal["pe"] += (4 - max(0, t)) * 2 * 27

                    LOOKAHEAD = 3
                    # at a segment start the S slots are free: put the first
                    # QKs ahead of the fillers so ACT restarts immediately
                    # (not at quarter 0's g0, whose drains are DMA-gated)
                    qk_first = not (j == 0 and g == 0)
                    for i in range(n_i):
                        if i < LOOKAHEAD and qk_first:
                            emit_qk(i)
                        if i >= LOOKAHEAD:
                            emit_av(i - LOOKAHEAD)
                        if g == 0 and i < 4 and v_units:
                            v_units.pop(0)()
                        # must-queue at slot rate
                        left = (2 - g) * n_i - i - 1
                        quota = (-(-len(queue) // max(1, left))
                                 if left else len(queue))
                        for _ in range(min(quota, 2, len(queue))):
                            queue.pop(0)()
                        # deferrables only into ACT-idle budget
                        while defq and bal["pe"] + 300 < bal["act"]:
                            pe_note(defq.pop(0)() or 0)
                        if i >= LOOKAHEAD or not qk_first:
                            emit_qk(i)
                    # ---- deferred normalize + transpose chain for (j, g);
                    # halves=(0,), (1,) or (0, 1) per unit pair ----
                    def make_norm(j=j, g=g, o_ps=o_ps, halves=(0, 1)):
                        onorm = {}

                        def norm():
                            key = (j, g)
                            if key not in onorm_sh:
                                onorm_sh[key] = onorm_pool.tile(
                                    [128, 4, 2, 64], BF16, tag="onorm",
                                    name=f"on{j}{g}")
                            on = onorm_sh[key]
                            onorm[0] = on
                            for half in halves:
                                rc = work.tile([128, 2, 2], F32, tag="recip",
                                               name=f"rc{j}{g}{half}")
                                nc.vector.reciprocal(
                                    rc[:], o_ps[half][:, :, :, 64])
                                nc.vector.tensor_mul(
                                    on[:, 2 * half : 2 * half + 2, :, :],
                                    o_ps[half][:, :, :, 0:64]
                                        .rearrange("p s u c -> p u s c"),
                                    rc.rearrange("p s u -> p u s")
                                        .unsqueeze(3)
                                        .broadcast_to([128, 2, 2, 64]),
                                )

                        def transp():
                            on = onorm[0]
                            for half in halves:
                                trp_h = mm.tile(
                                    [128, 2, 128], BF16, tag="mm",
                                    name=f"tr{j}{g}{half}")
                                for uu in range(2):
                                    u = 2 * half + uu
                                    nc.tensor.matmul(
                                        trp_h[:, uu, :],
                                        on[:, u, :, :],
                                        ident[:],
                                        start=True,
                                        stop=True,
                                        is_transpose=True,
                                    )
                                pe_note(2 * 53)
                                nc.vector.tensor_copy(
                                    onT[:, g, QS * j + 256 * half :
                                        QS * j + 256 * half + 256],
                                    trp_h.rearrange("p u q -> p (u q)"),
                                )

                        return [norm, transp]

                    if j == NQS - 1 and g == 1:
                        # tail: per-half chains interleave into the drain so
                        # the projection starts before the last exps finish
                        emit_av(n_i - LOOKAHEAD)      # stops u0, u1
                        for f in make_norm(halves=(0,)):
                            f()
                        for blk in (0, 1):
                            for nh in range(2):
                                for c in range(2):
                                    pu(blk, nh, c)()
                        emit_av(n_i - 2)
                        emit_av(n_i - 1)
                        for f in make_norm(halves=(1,)):
                            f()
                        for blk in (2, 3):
                            for nh in range(2):
                                for c in range(2):
                                    pu(blk, nh, c)()
                    else:
                        for i in range(max(0, n_i - LOOKAHEAD), n_i):
                            emit_av(i)
                        if g == 0:
                            # run in g1's segment: o slots recycle promptly
                            queue = make_norm() + queue
                        else:
                            pending.extend(make_norm())

                # flush must-fillers before the next quarter
                while queue:
                    queue.pop(0)()
                # defq leftovers carry forward
                carry = defq
                if j < NQS - 1:
                    pending.extend(pu(blk, nh, c) for blk in range(4)
                                   for nh in range(2) for c in range(2))



            # ---- tail: any carried deferrable units ----
            for f in carry:
                f()
            for f in pending:
                f()

    nc.compile()
    return nc


def _host_prep(x, W_qkv, b_qkv, W_proj, b_proj):
    """Build per-core input maps."""
    import ml_dtypes
    f8 = ml_dtypes.float8_e4m3
    bf = ml_dtypes.bfloat16

    x = np.asarray(x, dtype=np.float32)
    W_qkv = np.asarray(W_qkv, dtype=np.float32)
    b_qkv = np.asarray(b_qkv, dtype=np.float32)
    W_proj = np.asarray(W_proj, dtype=np.float32)

    x8s, xr8s = [], []
    for b in range(B):
        x4 = np.ascontiguousarray(x[b].T) * XS
        x8 = x4.astype(f8)
        xr8 = (x4 - x8.astype(np.float32)).astype(f8)
        x8s.append(x8)
        xr8s.append(xr8)
    ident = np.eye(128, dtype=np.float32).astype(bf)

    def pack_w(Wcols):
        """[1024, F] -> [128, KP, 2, F] (rows chunked 128*(2kp+two)+p)."""
        F = Wcols.shape[1]
        return Wcols.reshape(KP, 2, 128, F).transpose(2, 0, 1, 3)

    in_maps = []
    for c in range(NCORES):
        b, g4 = divmod(c, GROUPS)
        col0 = 256 * g4

        W8 = np.zeros((128, KP, 2, 4, 128), dtype=np.float32)
        Wr8 = np.zeros((128, KP, 2, 4, 128), dtype=np.float32)
        bqk = np.zeros((128, 4), dtype=np.float32)
        for t in range(4):
            qk, g = divmod(t, 2)
            c0 = 1024 * qk + col0 + 128 * g
            Ws = WS * W_qkv[:, c0 : c0 + 128]
            W8t = Ws.astype(f8).astype(np.float32)
            Wr8t = (Ws - W8t).astype(f8).astype(np.float32)
            W8[:, :, :, t, :] = pack_w(W8t)
            Wr8[:, :, :, t, :] = pack_w(Wr8t)
            bqk[:, t] = (XS * WS) * b_qkv[c0 : c0 + 128]
        W8 = np.ascontiguousarray(W8.reshape(128, -1)).astype(f8)
        Wr8 = np.ascontiguousarray(Wr8.reshape(128, -1)).astype(f8)

        Wvs = WS * W_qkv[:, 2048 + col0 : 2048 + col0 + 256]
        Wv8t = Wvs.astype(f8).astype(np.float32)
        Wvr8t = (Wvs - Wv8t).astype(f8).astype(np.float32)
        Wv8 = np.ascontiguousarray(pack_w(Wv8t).reshape(128, -1)).astype(f8)
        Wvr8 = np.ascontiguousarray(pack_w(Wvr8t).reshape(128, -1)).astype(f8)

        bv = b_qkv[2048 + col0 : 2048 + col0 + 256]
        vbias = np.ascontiguousarray(
            np.broadcast_to(bv, (128, 256))).astype(np.float32)
        Wp = np.ascontiguousarray(
            W_proj[col0 : col0 + 256].reshape(2, 128, D).transpose(1, 0, 2)
            .reshape(128, 2 * D)
        ).astype(bf)
        in_maps.append(
            {
                "x8": x8s[b],
                "xr8": xr8s[b],
                "W8": W8,
                "Wr8": Wr8,
                "Wv8": Wv8,
                "Wvr8": Wvr8,
                "Wp": Wp,
                "bqk": bqk,
                "vbias": vbias,
                "identD": ident,
            }
        )
    return in_maps


def _make_runner(nc):
    """Build the PJRT executable once (mirrors bass2jax.run_bass_via_pjrt)
    so repeated kernel() calls skip re-tracing/compile-cache lookups."""
    import jax
    from jax.sharding import Mesh, PartitionSpec
    from jax.experimental.shard_map import shard_map

    from concourse.bass2jax import (
        _bass_exec_p,
        install_neuronx_cc_hook,
        partition_id_tensor,
    )

    install_neuronx_cc_hook()
    partition_name = (
        nc.partition_id_tensor.name if nc.partition_id_tensor else None
    )
    in_names, out_names, out_avals, zero_outs = [], [], [], []
    for alloc in nc.m.functions[0].allocations:
        if not isinstance(alloc, mybir.MemoryLocationSet):
            continue
        name = alloc.memorylocations[0].name
        if alloc.kind == "ExternalInput":
            if name != partition_name:
                in_names.append(name)
        elif alloc.kind == "ExternalOutput":
            out_names.append(name)
            shape = tuple(alloc.tensor_shape)
            dtype = mybir.dt.np(alloc.dtype)
            out_avals.append(jax.core.ShapedArray(shape, dtype))
            zero_outs.append(np.zeros(shape, dtype))
    n_params = len(in_names)
    all_in_names = in_names + out_names
    if partition_name is not None:
        all_in_names = all_in_names + [partition_name]

    def _body(*args):
        operands = list(args)
        if partition_name is not None:
            operands.append(partition_id_tensor())
        return tuple(
            _bass_exec_p.bind(
                *operands,
                out_avals=tuple(out_avals),
                in_names=tuple(all_in_names),
                out_names=tuple(out_names),
                lowering_input_output_aliases=(),
                sim_require_finite=True,
                sim_require_nnan=True,
                nc=nc,
            )
        )

    devices = jax.devices()[:NCORES]
    mesh = Mesh(np.asarray(devices), ("core",))
    in_specs = (PartitionSpec("core"),) * (n_params + len(out_names))
    out_specs = (PartitionSpec("core"),) * len(out_names)
    fn = jax.jit(
        shard_map(_body, mesh=mesh, in_specs=in_specs,
                  out_specs=out_specs, check_rep=False),
        keep_unused=True,
    )
    concat_zeros = [
        np.zeros((NCORES * z.shape[0], *z.shape[1:]), z.dtype)
        for z in zero_outs
    ]

    def run(in_maps):
        concat_in = [
            np.concatenate([np.asarray(m[name]) for m in in_maps], axis=0)
            for name in in_names
        ]
        out_arrs = fn(*concat_in, *concat_zeros)
        return [
            {
                name: np.asarray(out_arrs[i]).reshape(
                    NCORES, *out_avals[i].shape
                )[c]
                for i, name in enumerate(out_names)
            }
            for c in range(NCORES)
        ]

    return run


def kernel(x, W_qkv, b_qkv, W_proj, b_proj):
    if "nc" not in _CACHE:
        _CACHE["nc"] = _build()
        try:
            _CACHE["run"] = _make_runner(_CACHE["nc"])
        except Exception:
            _CACHE["run"] = None
    in_maps = _host_prep(x, W_qkv, b_qkv, W_proj, b_proj)
    results = None
    if _CACHE["run"] is not None:
        try:
            results = _CACHE["run"](in_maps)
        except Exception:
            results = None
    if results is None:
        results = run_bass_kernel_spmd(
            _CACHE["nc"], in_maps, core_ids=list(range(NCORES))
        ).results
    out = np.zeros((B, N, D), dtype=np.float32)
    bp = np.asarray(b_proj, dtype=np.float32)
    for b in range(B):
        acc = results[4 * b]["y"].astype(np.float32).copy()
        for g in range(1, GROUPS):
            acc += results[4 * b + g]["y"]
        out[b] = acc + bp
    return out
